# revision 1
# baseline (speedup 1.0000x reference)
"""Distributed Bass/Tile kernel for nn_MessagePassing (radius-2 GNN) on 8 trn2 cores.

Strategy (graph/data parallel, per sharding hint):
  - Nodes sharded into 8 contiguous ranges of 12500 rows (padded to 12800/core).
  - Activations live TRANSPOSED in SBUF: [128 feat, rows] bf16.
  - Per step: two 2-layer MLPs computed shard-local (stationary-weight matmuls),
    the gather-table v written row-major bf16 and AllGathered across cores,
    then the edge gather (indirect DMA) + segment-sum via one-hot matmuls
    accumulating into PSUM over 64-row windows, + identity-matmul u-add.
  - Edge lists are presorted/padded on host per (core, level, 64-row window),
    with per-window chunk counts uniform across cores (compile-time program).
"""

import math
import numpy as np
import ml_dtypes

NCORES = 8
N = 100000
D = 128
NLEV = 4
RPC = N // NCORES          # 12500 rows owned per core
CH = 128                   # edges per chunk (matmul contract dim)
WIN = 64                   # one-hot window width (rows)
MT = 512                   # mlp/psum tile width (rows per matmul free dim)

BF16 = ml_dtypes.bfloat16

# step -> (u_set, u_j, v_set, v_j, level, u_src, v_src); set 0 = fc1, 1 = fc2
# src: 'l0','l1','l2' or 'x'
STEPS = [
    (0, 2, 0, 3, 3, "l2", "l2"),
    (0, 1, 1, 3, 2, "l1", "x"),
    (1, 1, 1, 2, 1, "x", "x"),
    (0, 0, 1, 0, 0, "l0", "x"),
]


def _round_up(a, b):
    return (a + b - 1) // b * b


def preprocess(features, edge_rows, edge_cols, edge_w, rpc=RPC, ncores=NCORES,
               win=WIN, ch=CH):
    """Host-side sharding. Returns (per_core_inputs, meta) where meta has the
    per-level chunk structure shared by all cores."""
    rpad = _round_up(rpc, MT)
    nwin = rpad // win
    edge_rows = np.asarray(edge_rows).astype(np.int64)
    edge_cols = np.asarray(edge_cols).astype(np.int64)
    edge_w = np.asarray(edge_w).astype(np.float32)
    nlev = edge_rows.shape[0]

    # per (level, core): sorted edge arrays; per level: uniform window chunk counts
    counts = np.zeros((nlev, ncores, nwin), np.int64)
    percore = [[None] * nlev for _ in range(ncores)]
    for L in range(nlev):
        rows, cols, ws = edge_rows[L], edge_cols[L], edge_w[L]
        owner = rows // rpc
        for c in range(ncores):
            m = owner == c
            r = rows[m] - c * rpc
            col = cols[m]
            wv = ws[m]
            wdx = r // win
            order = np.lexsort((col, wdx))
            r, col, wv, wdx = r[order], col[order], wv[order], wdx[order]
            percore[c][L] = (r, col, wv, wdx)
            counts[L, c] = np.bincount(wdx, minlength=nwin)

    # chunks per window: max over cores, ceil to chunks, >= 1
    cw = np.maximum(1, (counts.max(axis=1) + ch - 1) // ch)  # [nlev, nwin]
    nchunks = cw.sum(axis=1).astype(np.int64)                # [nlev]

    # build padded transposed metadata arrays per (core, level)
    per_core_inputs = [dict() for _ in range(ncores)]
    for L in range(nlev):
        nck = int(nchunks[L])
        starts = np.concatenate([[0], np.cumsum(cw[L])[:-1]])  # chunk offset per window
        for c in range(ncores):
            r, col, wv, wdx = percore[c][L]
            colp = np.zeros(nck * ch, np.int64)
            mp = np.zeros(nck * ch, np.int64)
            wp = np.zeros(nck * ch, np.float32)
            # place each window's edges at its chunk range
            wstart_e = np.concatenate([[0], np.cumsum(np.bincount(wdx, minlength=nwin))])
            for wdx_i in range(nwin):
                e0, e1 = wstart_e[wdx_i], wstart_e[wdx_i + 1]
                k = e1 - e0
                q0 = starts[wdx_i] * ch
                colp[q0:q0 + k] = col[e0:e1]
                mp[q0:q0 + k] = r[e0:e1] - wdx_i * win
                wp[q0:q0 + k] = wv[e0:e1]
                # padding stays (col 0, m 0, w 0)
            # adjust cols to padded global table indices
            colp = (colp // rpc) * rpad + (colp % rpc)
            # transpose to [128, nck] chunk layout: element q = k*128 + p -> [p, k]
            per_core_inputs[c][f"cols{L}"] = np.ascontiguousarray(
                colp.reshape(nck, ch).T.astype(np.int32))
            per_core_inputs[c][f"m{L}"] = np.ascontiguousarray(
                mp.reshape(nck, ch).T.astype(BF16))
            per_core_inputs[c][f"w{L}"] = np.ascontiguousarray(
                wp.reshape(nck, ch).T.astype(BF16))

    # features -> transposed, padded, bf16 per core
    features = np.asarray(features)
    for c in range(ncores):
        lt = np.zeros((features.shape[0], D, rpad), BF16)
        blk = features[:, c * rpc:(c + 1) * rpc, :]
        lt[:, :, :rpc] = np.transpose(blk, (0, 2, 1)).astype(BF16)
        per_core_inputs[c]["lT"] = lt

    meta = {
        "rpad": rpad,
        "nwin": nwin,
        "cw": cw,
        "nchunks": nchunks,
    }
    return per_core_inputs, meta


def pack_weights(fc1_W1, fc1_b1, fc1_W2, fc1_b2, fc2_W1, fc2_b1, fc2_W2, fc2_b2):
    """Returns weight input dict (same for all cores)."""
    W1 = [np.asarray(fc1_W1), np.asarray(fc2_W1)]
    W2 = [np.asarray(fc1_W2), np.asarray(fc2_W2)]
    b1 = [np.asarray(fc1_b1), np.asarray(fc2_b1)]
    b2 = [np.asarray(fc1_b2), np.asarray(fc2_b2)]
    wk = np.zeros((16, D, D), BF16)
    bias1 = np.zeros((8, D, 1), np.float32)     # [step*2 + (0=u,1=v)]
    bias2u = np.zeros((4, D, 1), np.float32)
    bias2v = np.zeros((4, D, D), np.float32)    # broadcast over rows (partition dim)
    for s, (us, uj, vs, vj, _L, _usrc, _vsrc) in enumerate(STEPS):
        wk[4 * s + 0] = W1[us][uj].astype(BF16)
        wk[4 * s + 1] = W2[us][uj].astype(BF16)
        wk[4 * s + 2] = W1[vs][vj].astype(BF16)
        wk[4 * s + 3] = W2[vs][vj].astype(BF16)
        bias1[2 * s + 0, :, 0] = b1[us][uj]
        bias1[2 * s + 1, :, 0] = b1[vs][vj]
        bias2u[s, :, 0] = b2[us][uj]
        bias2v[s] = np.broadcast_to(b2[vs][vj][None, :], (D, D))
    return {"Wk": wk, "Bias1": bias1, "Bias2u": bias2u, "Bias2v": bias2v}


def build_program(meta, ncores=NCORES, gather_bufs=24):
    """Builds the Bacc program (single SPMD program for all cores)."""
    from contextlib import ExitStack
    import concourse.bass as bass
    import concourse.tile as tile
    import concourse.mybir as mybir
    from concourse import bacc

    dt = mybir.dt
    rpad = meta["rpad"]
    nwin = meta["nwin"]
    cw = meta["cw"]
    nchunks = meta["nchunks"]
    nfull = ncores * rpad
    nbt = rpad // MT            # psum/mlp blocks per core
    nrt = rpad // CH            # 128-row tiles per core
    wpb = MT // WIN             # windows per block

    nc = bacc.Bacc(None, target_bir_lowering=False, num_devices=ncores)

    lT = nc.dram_tensor("lT", [3, D, rpad], dt.bfloat16, kind="ExternalInput")
    Wk = nc.dram_tensor("Wk", [16, D, D], dt.bfloat16, kind="ExternalInput")
    Bias1 = nc.dram_tensor("Bias1", [8, D, 1], dt.float32, kind="ExternalInput")
    Bias2u = nc.dram_tensor("Bias2u", [4, D, 1], dt.float32, kind="ExternalInput")
    Bias2v = nc.dram_tensor("Bias2v", [4, D, D], dt.float32, kind="ExternalInput")
    colsL, mL, wL = [], [], []
    for L in range(NLEV):
        nck = int(nchunks[L])
        colsL.append(nc.dram_tensor(f"cols{L}", [CH, nck], dt.int32, kind="ExternalInput"))
        mL.append(nc.dram_tensor(f"m{L}", [CH, nck], dt.bfloat16, kind="ExternalInput"))
        wL.append(nc.dram_tensor(f"w{L}", [CH, nck], dt.bfloat16, kind="ExternalInput"))
    y_out = nc.dram_tensor("y_out", [D, rpad], dt.float32, kind="ExternalOutput")

    agi = [nc.dram_tensor(f"agi{p}", [rpad, D], dt.bfloat16) for p in range(2)]
    ago = [nc.dram_tensor(f"ago{p}", [nfull, D], dt.bfloat16, addr_space="Shared")
           for p in range(2)]

    with tile.TileContext(nc) as tc:
        with ExitStack() as ctx:
            const_p = ctx.enter_context(tc.tile_pool(name="const", bufs=1))
            wpool = ctx.enter_context(tc.tile_pool(name="wpool", bufs=2))
            xpool = ctx.enter_context(tc.tile_pool(name="xpool", bufs=2))
            upool = ctx.enter_context(tc.tile_pool(name="upool", bufs=2))
            lpool = ctx.enter_context(tc.tile_pool(name="lpool", bufs=1))
            vpool = ctx.enter_context(tc.tile_pool(name="vpool", bufs=1))
            hpool = ctx.enter_context(tc.tile_pool(name="hpool", bufs=3))
            spool = ctx.enter_context(tc.tile_pool(name="spool", bufs=2))
            mpool = ctx.enter_context(tc.tile_pool(name="mpool", bufs=2))
            gpool = ctx.enter_context(tc.tile_pool(name="gpool", bufs=gather_bufs))
            ypool = ctx.enter_context(tc.tile_pool(name="ypool", bufs=3))
            ps_mlp = ctx.enter_context(tc.tile_pool(name="ps_mlp", bufs=2, space="PSUM"))
            ps_seg = ctx.enter_context(tc.tile_pool(name="ps_seg", bufs=2, space="PSUM"))
            ps_v = ctx.enter_context(tc.tile_pool(name="ps_v", bufs=2, space="PSUM"))

            # constants
            iota_i = const_p.tile([CH, WIN], dt.int32)
            nc.gpsimd.iota(iota_i[:], pattern=[[1, WIN]], base=0, channel_multiplier=0)
            iota64 = const_p.tile([CH, WIN], dt.bfloat16)
            nc.vector.tensor_copy(iota64[:], iota_i[:])
            ident = const_p.tile([CH, CH], dt.bfloat16)
            from concourse.masks import make_identity
            make_identity(nc, ident[:])

            def load_weights(s):
                w = []
                for k in range(4):
                    t = wpool.tile([D, D], dt.bfloat16, tag=f"w{k}")
                    nc.sync.dma_start(out=t[:], in_=Wk[4 * s + k])
                    w.append(t)
                b1u = wpool.tile([D, 1], dt.float32, tag="b1u")
                nc.sync.dma_start(out=b1u[:], in_=Bias1[2 * s + 0])
                b1v = wpool.tile([D, 1], dt.float32, tag="b1v")
                nc.sync.dma_start(out=b1v[:], in_=Bias1[2 * s + 1])
                b2u = wpool.tile([D, 1], dt.float32, tag="b2u")
                nc.sync.dma_start(out=b2u[:], in_=Bias2u[s])
                b2v = wpool.tile([D, D], dt.float32, tag="b2v")
                nc.sync.dma_start(out=b2v[:], in_=Bias2v[s])
                return w, b1u, b1v, b2u, b2v

            def mlp_transposed(src, W1t, b1t, W2t, b2t):
                """u_T = W2^T relu(W1^T src + b1) + b2, all [128, rpad] bf16."""
                u_t = upool.tile([D, rpad], dt.bfloat16, tag="u")
                for t in range(nbt):
                    sl = slice(t * MT, (t + 1) * MT)
                    hp = ps_mlp.tile([D, MT], dt.float32, tag="mlp")
                    nc.tensor.matmul(hp[:], lhsT=W1t[:], rhs=src[:, sl],
                                     start=True, stop=True)
                    ht = hpool.tile([D, MT], dt.bfloat16, tag="h")
                    nc.scalar.activation(ht[:], hp[:],
                                         mybir.ActivationFunctionType.Relu,
                                         bias=b1t[:], scale=1.0)
                    up = ps_mlp.tile([D, MT], dt.float32, tag="mlp")
                    nc.tensor.matmul(up[:], lhsT=W2t[:], rhs=ht[:],
                                     start=True, stop=True)
                    nc.vector.tensor_scalar(u_t[:, sl], up[:], b2t[:], None,
                                            mybir.AluOpType.add)
                return u_t

            def mlp_rowmajor_to_dram(src, W1t, b1t, W2t, b2vt, dram_dst):
                """v = relu(src^T W1 + b1) W2 + b2 written row-major to dram."""
                v_sb = vpool.tile([CH, nrt * D], dt.bfloat16, tag="v")
                for t in range(nbt):
                    sl = slice(t * MT, (t + 1) * MT)
                    hp = ps_mlp.tile([D, MT], dt.float32, tag="mlp")
                    nc.tensor.matmul(hp[:], lhsT=W1t[:], rhs=src[:, sl],
                                     start=True, stop=True)
                    ht = hpool.tile([D, MT], dt.bfloat16, tag="h")
                    nc.scalar.activation(ht[:], hp[:],
                                         mybir.ActivationFunctionType.Relu,
                                         bias=b1t[:], scale=1.0)
                    for q in range(MT // CH):
                        r = t * (MT // CH) + q
                        vp = ps_v.tile([CH, D], dt.float32, tag="vps")
                        nc.tensor.matmul(vp[:], lhsT=ht[:, q * CH:(q + 1) * CH],
                                         rhs=W2t[:], start=True, stop=True)
                        nc.vector.tensor_tensor(
                            out=v_sb[:, r * D:(r + 1) * D], in0=vp[:], in1=b2vt[:],
                            op=mybir.AluOpType.add)
                nc.sync.dma_start(
                    out=dram_dst[:].rearrange("(t p) f -> p t f", p=CH),
                    in_=v_sb[:].rearrange("p (t f) -> p t f", f=D))
                return v_sb

            x_cur = None
            l_cache = {}

            def get_src(name, x_cur):
                if name == "x":
                    return x_cur
                idx = int(name[1])
                t = lpool.tile([D, rpad], dt.bfloat16, tag="l")
                nc.sync.dma_start(out=t[:], in_=lT[idx])
                return t

            for s, (_us, _uj, _vs, _vj, L, usrc, vsrc) in enumerate(STEPS):
                w4, b1u, b1v, b2u, b2v = load_weights(s)
                src_u = get_src(usrc, x_cur)
                src_v = src_u if vsrc == usrc else get_src(vsrc, x_cur)
                u_t = mlp_transposed(src_u, w4[0], b1u, w4[1], b2u)
                mlp_rowmajor_to_dram(src_v, w4[2], b1v, w4[3], b2v, agi[s % 2])
                nc.gpsimd.collective_compute(
                    "AllGather", mybir.AluOpType.bypass,
                    replica_groups=[list(range(ncores))],
                    ins=[agi[s % 2][:]], outs=[ago[s % 2][:]],
                )
                vtab = ago[s % 2]

                final = s == len(STEPS) - 1
                if not final:
                    x_next = xpool.tile([D, rpad], dt.bfloat16, tag="x")

                cwl = cw[L]
                chunk0 = 0
                for b in range(nbt):
                    ps = ps_seg.tile([D, MT], dt.float32, tag="seg")
                    cb = int(cwl[b * wpb:(b + 1) * wpb].sum())
                    # metadata + S build for the whole block
                    mt = mpool.tile([CH, cb], dt.bfloat16, tag="m")
                    nc.sync.dma_start(out=mt[:], in_=mL[L][:, chunk0:chunk0 + cb])
                    wt = mpool.tile([CH, cb], dt.bfloat16, tag="w")
                    nc.sync.dma_start(out=wt[:], in_=wL[L][:, chunk0:chunk0 + cb])
                    ct = mpool.tile([CH, cb], dt.int32, tag="c")
                    nc.sync.dma_start(out=ct[:], in_=colsL[L][:, chunk0:chunk0 + cb])
                    st = spool.tile([CH, cb * WIN], dt.bfloat16, tag="s")
                    s3 = st[:].rearrange("p (c j) -> p c j", j=WIN)
                    nc.vector.tensor_tensor(
                        out=s3, in0=iota64[:].unsqueeze(1).to_broadcast([CH, cb, WIN]),
                        in1=mt[:].unsqueeze(2).to_broadcast([CH, cb, WIN]),
                        op=mybir.AluOpType.is_equal)
                    nc.vector.tensor_tensor(
                        out=s3, in0=s3,
                        in1=wt[:].unsqueeze(2).to_broadcast([CH, cb, WIN]),
                        op=mybir.AluOpType.mult)
                    k = 0
                    for wi in range(wpb):
                        cwk = int(cwl[b * wpb + wi])
                        for j in range(cwk):
                            g = gpool.tile([CH, D], dt.bfloat16, tag="g")
                            nc.gpsimd.indirect_dma_start(
                                out=g[:], out_offset=None, in_=vtab[:],
                                in_offset=bass.IndirectOffsetOnAxis(
                                    ap=ct[:, k:k + 1], axis=0))
                            nc.tensor.matmul(
                                ps[:, wi * WIN:(wi + 1) * WIN], lhsT=g[:],
                                rhs=st[:, k * WIN:(k + 1) * WIN],
                                start=(k == 0), stop=(j == cwk - 1),
                                skip_group_check=True)
                            k += 1
                    chunk0 += cb
                    # u add + flush
                    sl = slice(b * MT, (b + 1) * MT)
                    nc.tensor.matmul(ps[:], lhsT=ident[:], rhs=u_t[:, sl],
                                     start=False, stop=True, skip_group_check=True)
                    if final:
                        yt = ypool.tile([D, MT], dt.float32, tag="y")
                        nc.vector.tensor_copy(yt[:], ps[:])
                        nc.sync.dma_start(out=y_out[:, sl], in_=yt[:])
                    else:
                        nc.vector.tensor_copy(x_next[:, sl], ps[:])
                if not final:
                    x_cur = x_next

    nc.compile()
    return nc


_CACHE = {}


def make_runner(nc, ncores=NCORES):
    """Builds a reusable jitted SPMD executor for the program (jit once)."""
    import jax
    import numpy as np
    from jax.experimental.shard_map import shard_map
    from jax.sharding import Mesh, PartitionSpec
    from concourse import bass2jax

    bass2jax.install_neuronx_cc_hook()
    import concourse.mybir as mybir

    partition_name = nc.partition_id_tensor.name if nc.partition_id_tensor else None
    in_names, out_names, out_avals, zero_outs = [], [], [], []
    for alloc in nc.m.functions[0].allocations:
        if not isinstance(alloc, mybir.MemoryLocationSet):
            continue
        name = alloc.memorylocations[0].name
        if alloc.kind == "ExternalInput":
            if name != partition_name:
                in_names.append(name)
        elif alloc.kind == "ExternalOutput":
            out_names.append(name)
            shape = tuple(alloc.tensor_shape)
            dtype = mybir.dt.np(alloc.dtype)
            out_avals.append(jax.core.ShapedArray(shape, dtype))
            zero_outs.append(np.zeros(shape, dtype))
    n_params = len(in_names)

    def _body(*args):
        operands = list(args)
        if partition_name is not None:
            operands.append(bass2jax.partition_id_tensor())
        outs = bass2jax._bass_exec_p.bind(
            *operands,
            out_avals=tuple(out_avals),
            in_names=tuple(in_names + out_names +
                           ([partition_name] if partition_name else [])),
            out_names=tuple(out_names),
            lowering_input_output_aliases=(),
            sim_require_finite=True,
            sim_require_nnan=True,
            nc=nc,
        )
        return tuple(outs)

    devices = jax.devices()[:ncores]
    mesh = Mesh(np.asarray(devices), ("core",))
    n_outs = len(out_names)
    sharded = jax.jit(
        shard_map(_body, mesh=mesh,
                  in_specs=(PartitionSpec("core"),) * (n_params + n_outs),
                  out_specs=(PartitionSpec("core"),) * n_outs,
                  check_rep=False),
        keep_unused=True,
    )

    def run(in_maps, iters=1):
        import time
        concat_in = [
            np.concatenate([np.asarray(in_maps[c][name]) for c in range(ncores)], axis=0)
            for name in in_names
        ]
        concat_zeros = [
            np.zeros((ncores * z.shape[0], *z.shape[1:]), z.dtype) for z in zero_outs
        ]
        args = [jax.device_put(a) for a in concat_in + concat_zeros]
        out = sharded(*args)
        jax.block_until_ready(out)
        times = []
        for _ in range(max(0, iters - 1)):
            t0 = time.perf_counter()
            out = sharded(*args)
            jax.block_until_ready(out)
            times.append(time.perf_counter() - t0)
        results = [
            {name: np.asarray(out[i]).reshape(ncores, *out_avals[i].shape)[c]
             for i, name in enumerate(out_names)}
            for c in range(ncores)
        ]
        return results, times

    return run


def _run(per_core_inputs, weights, meta, iters=1):
    key = tuple(int(x) for x in meta["nchunks"]) + (meta["rpad"],)
    if key not in _CACHE:
        nc = build_program(meta)
        _CACHE[key] = make_runner(nc)
    run = _CACHE[key]
    in_maps = [dict(ci, **weights) for ci in per_core_inputs]
    return run(in_maps, iters=iters)


def kernel(features, edge_rows, edge_cols, edge_w,
           fc1_W1, fc1_b1, fc1_W2, fc1_b2,
           fc2_W1, fc2_b1, fc2_W2, fc2_b2):
    per_core_inputs, meta = preprocess(features, edge_rows, edge_cols, edge_w)
    weights = pack_weights(fc1_W1, fc1_b1, fc1_W2, fc1_b2,
                           fc2_W1, fc2_b1, fc2_W2, fc2_b2)
    results, _times = _run(per_core_inputs, weights, meta)
    out = np.empty((N, D), np.float32)
    for c in range(NCORES):
        yt = results[c]["y_out"]              # [D, rpad] fp32
        out[c * RPC:(c + 1) * RPC] = yt.T[:RPC]
    return out

